# revision 26
# baseline (speedup 1.0000x reference)
"""Trainium2 Bass kernel for nn_Block dense_cnn problem.

Computation (B=128, T=512, C=1024):
    h = x @ W_proj.T ; v = h @ W_values.T
    z[c,t] = line[t] ** (2 + sigmoid(pow_[c]) * 100)       (power-law kernel)
    y[b,c,:] = causal_conv(z[c,:], v[b,:,c])               (FFT conv in reference)
    out = relu(y * gain)

Strategy: pure data parallelism over batch across 8 NeuronCores (16
batches per core).  The two dense projections fold into one combined
matmul (Wc = W_values @ W_proj, f64 on host).  The causal conv runs as
chunked Toeplitz matmuls on the TensorEngine: time is split into 4
chunks of 128; for diagonal offset d and channel c,
G[d][c][s][t] = gain[c] * z[c, 128*d + t - s]  (zero for negative lag),
and the conv accumulates over d in PSUM.  gain is folded into G, so the
epilogue is just relu.

Weight-derived constants (WcT, G) are built on host once and cached on
device, keyed on the raw bytes of the weight inputs; per call only x is
uploaded (bf16) and y downloaded (bf16).  The Bass kernel is embedded
in a cached jax.jit/shard_map callable via the bass_exec custom call,
so there is no per-call retrace or reload.
"""

import hashlib
import numpy as np

import ml_dtypes

B, T, C = 128, 512, 1024
NCORES = 8
B_LOC = B // NCORES            # 16 batches per core
R_LOC = B_LOC * T              # 8192 rows per core
CH = 128                       # time chunk
ND = T // CH                   # 4 chunks
NK = C // 128                  # 8 contraction blocks
CG = 16                        # channels per G slab
YG = 128                       # channels per output staging block

_cache = {}


def _sigmoid(u):
    return 1.0 / (1.0 + np.exp(-u))


def _build_consts(W_proj, W_values, gain, pow_, line):
    """Host-side build of the combined weight and Toeplitz tables."""
    bf16 = ml_dtypes.bfloat16
    Wc = W_values.astype(np.float64) @ W_proj.astype(np.float64)   # (d, c)
    WcT = np.ascontiguousarray(Wc.T).astype(bf16)                  # (c_in=k, d)

    p = 2.0 + _sigmoid(pow_.reshape(C).astype(np.float64)) * 100.0
    ln = line.reshape(T).astype(np.float64)
    z = ln[None, :] ** p[:, None]                                  # (C, T)
    z = z * gain.reshape(C).astype(np.float64)[:, None]            # fold gain
    z = z.astype(np.float32)

    s_idx = np.arange(CH)
    t_idx = np.arange(CH)
    # G layout: [d][s][c][t] so a (d, c-group) slab is a 2D-contiguous DMA
    G = np.zeros((ND, CH, C, CH), np.float32)
    for d in range(ND):
        lag = CH * d + t_idx[None, :] - s_idx[:, None]             # (s, t)
        valid = lag >= 0
        lag_c = np.clip(lag, 0, T - 1)
        Gd = z[:, lag_c]                                           # (c, s, t)
        Gd[:, ~valid] = 0.0
        G[d] = Gd.transpose(1, 0, 2)                               # (s, c, t)
    return WcT, G.astype(bf16)


def _build_bass(nd_c=None):
    """nd_c: per-channel number of Toeplitz diagonal blocks to keep
    (1..ND); None keeps all ND for every channel."""
    from concourse import bacc, mybir, tile

    if nd_c is None:
        nd_c = [ND] * C
    nc = bacc.Bacc("TRN2", target_bir_lowering=False, debug=False)
    bf = mybir.dt.bfloat16
    f32 = mybir.dt.float32

    x_in = nc.dram_tensor("x", [R_LOC, C], bf, kind="ExternalInput")
    wct_in = nc.dram_tensor("wct", [C, C], bf, kind="ExternalInput")
    g_in = nc.dram_tensor("g", [ND, CH, C, CH], bf, kind="ExternalInput")
    y_out = nc.dram_tensor("y", [R_LOC, C], bf, kind="ExternalOutput")

    with tile.TileContext(nc) as tc:
        with tc.tile_pool(name="vpool", bufs=1) as vpool:
            # v[s, (b j), c] : stage-A output, conv input.  128 KiB/partition.
            v_sb = vpool.tile([CH, B_LOC * ND, C], bf)

            # ---- Stage A: v = x @ Wc^T ----
            with (
                tc.tile_pool(name="wct", bufs=1) as wctp,
                tc.tile_pool(name="xt", bufs=2) as xtp,
                tc.tile_pool(name="psA", bufs=4, space="PSUM") as psA,
            ):
                wct_sb = wctp.tile([128, NK, C], bf)
                # wct (k_blk*128 + k, d) -> sbuf [k, k_blk, d]
                nc.scalar.dma_start(
                    out=wct_sb[:],
                    in_=wct_in[:].rearrange("(kb k) d -> k kb d", k=128),
                )
                for b in range(B_LOC):
                    # x^T tiles for this batch: [k, k_blk, s(512)]
                    xt = xtp.tile([128, NK, T], bf)
                    for kb in range(NK):
                        nc.sync.dma_start_transpose(
                            xt[:, kb, :],
                            x_in[b * T:(b + 1) * T, kb * 128:(kb + 1) * 128],
                        )
                    for j in range(ND):
                        for half in range(2):
                            ps = psA.tile([128, 512], f32)
                            for kb in range(NK):
                                nc.tensor.matmul(
                                    ps[:],
                                    xt[:, kb, j * 128:(j + 1) * 128],
                                    wct_sb[:, kb, half * 512:(half + 1) * 512],
                                    start=(kb == 0),
                                    stop=(kb == NK - 1),
                                )
                            nc.vector.tensor_copy(
                                v_sb[:, j * B_LOC + b, half * 512:(half + 1) * 512],
                                ps[:],
                            )

            # ---- Stage B: per-channel chunked Toeplitz conv ----
            with (
                tc.tile_pool(name="gsl", bufs=2) as gp,
                tc.tile_pool(name="ysb", bufs=2) as yp,
                tc.tile_pool(name="psB", bufs=4, space="PSUM") as psB,
            ):
                for c0 in range(0, C, YG):          # output staging block
                    y_sb = yp.tile([CH, B_LOC * ND, YG], bf)
                    for cg0 in range(c0, c0 + YG, CG):   # G slab group
                        nd_grp = max(nd_c[cg0:cg0 + CG])
                        gsl = gp.tile([CH, ND, CG, CH], bf)
                        for d in range(nd_grp):
                            nc.scalar.dma_start(
                                out=gsl[:, d, :, :],
                                in_=g_in[d, :, cg0:cg0 + CG, :],
                            )
                        for c4 in range(0, CG, 4):
                            # psum columns are j-major: col = j*B_LOC + b
                            ps = psB.tile([128, 4, ND * B_LOC], f32)
                            for ci in range(4):
                                c = cg0 + c4 + ci
                                nd = nd_c[c]
                                for d in range(nd):
                                    nc.tensor.matmul(
                                        ps[:, ci, d * B_LOC:ND * B_LOC],
                                        gsl[:, d, (c4 + ci), :],
                                        v_sb[:, 0:(ND - d) * B_LOC, c],
                                        start=(d == 0),
                                        stop=(d == nd - 1),
                                    )
                            # relu + cast, psum (t, ci, (j b)) -> y_sb (t, (b j), c)
                            co = cg0 - c0 + c4
                            nc.vector.tensor_scalar_max(
                                y_sb[:, :, co:co + 4].rearrange(
                                    "t (b j) ci -> t ci b j", j=ND),
                                ps[:].rearrange(
                                    "t ci (j b) -> t ci b j", b=B_LOC),
                                0.0,
                            )
                    # y_sb (t, (b j), c) -> y[(b, j*128+t), c0:c0+YG]
                    nc.scalar.dma_start(
                        out=y_out[:, c0:c0 + YG].rearrange(
                            "(b j t) c -> t (b j) c", t=CH, j=ND),
                        in_=y_sb[:],
                    )

    nc.finalize()
    return nc


def _make_runner(nd_c=None):
    """Build the Bass kernel and a cached jitted SPMD callable."""
    import jax
    import jax.numpy as jnp
    from jax.sharding import Mesh, PartitionSpec as P, NamedSharding
    from jax.experimental.shard_map import shard_map
    from concourse import bass2jax
    from concourse.bass2jax import (
        _bass_exec_p, install_neuronx_cc_hook, partition_id_tensor)

    install_neuronx_cc_hook()
    nc = _build_bass(nd_c)

    devs = jax.devices()[:NCORES]
    mesh = Mesh(np.asarray(devs), ("core",))

    out_aval = jax.core.ShapedArray((R_LOC, C), jnp.bfloat16)

    def _body(xb, wct, g, yz):
        outs = _bass_exec_p.bind(
            xb, wct, g, yz, partition_id_tensor(),
            out_avals=(out_aval,),
            in_names=("x", "wct", "g", "y", "partition_id"),
            out_names=("y",),
            lowering_input_output_aliases=(),
            sim_require_finite=False,
            sim_require_nnan=False,
            nc=nc,
        )
        return outs[0]

    mapped = jax.jit(shard_map(
        _body, mesh=mesh,
        in_specs=(P("core"),) * 4,
        out_specs=P("core"),
        check_rep=False,
    ), donate_argnums=(3,), keep_unused=True)

    zero_fn = jax.jit(
        lambda: jnp.zeros((B * T, C), jnp.bfloat16),
        out_shardings=NamedSharding(mesh, P("core")))
    return mapped, mesh, zero_fn


def _get_state(W_proj, W_values, gain, pow_, line):
    import jax
    from jax.sharding import PartitionSpec as P, NamedSharding

    h = hashlib.md5()
    for a in (W_proj, W_values, gain, pow_, line):
        h.update(np.ascontiguousarray(a).tobytes())
    key = h.hexdigest()
    if key in _cache:
        return _cache[key]

    # per-channel kept diagonal-block count: drop tail blocks whose
    # biggest kernel value is < 1e-4 of the channel peak (z[c,0] = 1)
    p = 2.0 + _sigmoid(np.asarray(pow_, np.float64).reshape(C)) * 100.0
    ln = np.asarray(line, np.float64).reshape(T)
    nd_c = []
    for c in range(C):
        nd = 1
        for d in range(1, ND):
            if ln[CH * d - 127] ** p[c] >= 1e-4:
                nd = d + 1
        nd_c.append(nd)

    rkey = ("runner", tuple(nd_c))
    if rkey not in _cache:
        _cache[rkey] = _make_runner(nd_c)
    mapped, mesh, zero_fn = _cache[rkey]

    WcT, G = _build_consts(
        np.asarray(W_proj, np.float32), np.asarray(W_values, np.float32),
        np.asarray(gain, np.float32), np.asarray(pow_, np.float32),
        np.asarray(line, np.float32))
    # stack per-core copies on axis 0 so each shard is exactly the
    # per-core BIR shape (no reshape inside the partitioned body)
    sh = NamedSharding(mesh, P("core"))
    wct_d = jax.device_put(
        np.ascontiguousarray(np.broadcast_to(WcT, (NCORES,) + WcT.shape))
        .reshape(NCORES * WcT.shape[0], WcT.shape[1]), sh)
    g_d = jax.device_put(
        np.ascontiguousarray(np.broadcast_to(G, (NCORES,) + G.shape))
        .reshape(NCORES * G.shape[0], *G.shape[1:]), sh)
    state = (mapped, mesh, zero_fn, wct_d, g_d)
    _cache[key] = state
    return state


def kernel(x, W_proj, W_values, gain, pow_, line):
    import jax
    from jax.sharding import PartitionSpec as P, NamedSharding

    mapped, mesh, zero_fn, wct_d, g_d = _get_state(
        W_proj, W_values, gain, pow_, line)

    xb = np.asarray(x, np.float32).reshape(B * T, C).astype(ml_dtypes.bfloat16)
    xd = jax.device_put(xb, NamedSharding(mesh, P("core")))
    out = mapped(xd, wct_d, g_d, zero_fn())
    y = np.asarray(out).astype(np.float32)
    return y.reshape(B, T, C)


# revision 37
# speedup vs baseline: 1.0753x; 1.0753x over previous
"""Trainium2 Bass kernel for nn_Block dense_cnn problem.

Computation (B=128, T=512, C=1024):
    h = x @ W_proj.T ; v = h @ W_values.T
    z[c,t] = line[t] ** (2 + sigmoid(pow_[c]) * 100)       (power-law kernel)
    y[b,c,:] = causal_conv(z[c,:], v[b,:,c])               (FFT conv in reference)
    out = relu(y * gain)

Strategy: pure data parallelism over batch across 8 NeuronCores (16
batches per core).  The two dense projections fold into one combined
matmul (Wc = W_values @ W_proj, f64 on host).  The causal conv runs as
chunked Toeplitz matmuls on the TensorEngine: time is split into 4
chunks of 128; for diagonal offset d and channel c,
G[d][c][s][t] = gain[c] * z[c, 128*d + t - s]  (zero for negative lag),
and the conv accumulates over d in PSUM.  gain is folded into G, so the
epilogue is just relu.

Weight-derived constants (WcT, G) are built on host once and cached on
device, keyed on the raw bytes of the weight inputs; per call only x is
uploaded (bf16) and y downloaded (bf16).  The Bass kernel is embedded
in a cached jax.jit/shard_map callable via the bass_exec custom call,
so there is no per-call retrace or reload.
"""

import hashlib
import numpy as np

import ml_dtypes

B, T, C = 128, 512, 1024
NCORES = 8
B_LOC = B // NCORES            # 16 batches per core
R_LOC = B_LOC * T              # 8192 rows per core
CH = 128                       # time chunk
ND = T // CH                   # 4 chunks
NK = C // 128                  # 8 contraction blocks
CG = 16                        # channels per G slab
YG = 128                       # channels per output staging block

_cache = {}


def _sigmoid(u):
    return 1.0 / (1.0 + np.exp(-u))


def _build_consts(W_proj, W_values, gain, pow_, line):
    """Host-side build of the combined weight and Toeplitz tables."""
    bf16 = ml_dtypes.bfloat16
    Wc = W_values.astype(np.float64) @ W_proj.astype(np.float64)   # (d, c)
    WcT = np.ascontiguousarray(Wc.T).astype(bf16)                  # (c_in=k, d)

    p = 2.0 + _sigmoid(pow_.reshape(C).astype(np.float64)) * 100.0
    ln = line.reshape(T).astype(np.float64)
    z = ln[None, :] ** p[:, None]                                  # (C, T)
    z = z * gain.reshape(C).astype(np.float64)[:, None]            # fold gain
    z = z.astype(np.float32)

    s_idx = np.arange(CH)
    t_idx = np.arange(CH)
    # G layout: [d][s][c][t] so a (d, c-group) slab is a 2D-contiguous DMA
    G = np.zeros((ND, CH, C, CH), np.float32)
    for d in range(ND):
        lag = CH * d + t_idx[None, :] - s_idx[:, None]             # (s, t)
        valid = lag >= 0
        lag_c = np.clip(lag, 0, T - 1)
        Gd = z[:, lag_c]                                           # (c, s, t)
        Gd[:, ~valid] = 0.0
        G[d] = Gd.transpose(1, 0, 2)                               # (s, c, t)
    return WcT, G.astype(bf16)


def _build_bass(nd_c=None):
    """nd_c: per-channel number of Toeplitz diagonal blocks to keep
    (1..ND); None keeps all ND for every channel."""
    from concourse import bacc, mybir, tile

    if nd_c is None:
        nd_c = [ND] * C
    nc = bacc.Bacc("TRN2", target_bir_lowering=False, debug=False)
    bf = mybir.dt.bfloat16
    f32 = mybir.dt.float32

    x_in = nc.dram_tensor("x", [R_LOC, C], bf, kind="ExternalInput")
    wct_in = nc.dram_tensor("wct", [C, C], bf, kind="ExternalInput")
    g_in = nc.dram_tensor("g", [ND, CH, C, CH], bf, kind="ExternalInput")
    y_out = nc.dram_tensor("y", [R_LOC, C], bf, kind="ExternalOutput")

    with tile.TileContext(nc) as tc:
        with tc.tile_pool(name="vpool", bufs=1) as vpool:
            # v[s, (b j), c] : stage-A output, conv input.  128 KiB/partition.
            v_sb = vpool.tile([CH, B_LOC * ND, C], bf)

            # ---- Stage A: v = x @ Wc^T ----
            with (
                tc.tile_pool(name="wct", bufs=1) as wctp,
                tc.tile_pool(name="xt", bufs=2) as xtp,
                tc.tile_pool(name="psA", bufs=2, space="PSUM") as psA,
            ):
                wct_sb = wctp.tile([128, NK, C], bf)
                # wct (k_blk*128 + k, d) -> sbuf [k, k_blk, d]
                nc.scalar.dma_start(
                    out=wct_sb[:],
                    in_=wct_in[:].rearrange("(kb k) d -> k kb d", k=128),
                )
                for b in range(B_LOC):
                    # x^T tiles for this batch: [k, k_blk, s(512)]
                    xt = xtp.tile([128, NK, T], bf)
                    for kb in range(NK):
                        nc.sync.dma_start_transpose(
                            xt[:, kb, :],
                            x_in[b * T:(b + 1) * T, kb * 128:(kb + 1) * 128],
                        )
                    for j in range(ND):
                        # half-inner so the second matmul reuses the
                        # loaded x-tile weights (no redundant LDWEIGHTS)
                        ps0 = psA.tile([128, 512], f32, tag="psA0")
                        ps1 = psA.tile([128, 512], f32, tag="psA1")
                        for kb in range(NK):
                            xw = xt[:, kb, j * 128:(j + 1) * 128]
                            nc.tensor.matmul(
                                ps0[:], xw, wct_sb[:, kb, 0:512],
                                start=(kb == 0), stop=(kb == NK - 1))
                            nc.tensor.matmul(
                                ps1[:], xw, wct_sb[:, kb, 512:1024],
                                start=(kb == 0), stop=(kb == NK - 1))
                        nc.vector.tensor_copy(
                            v_sb[:, j * B_LOC + b, 0:512], ps0[:])
                        nc.vector.tensor_copy(
                            v_sb[:, j * B_LOC + b, 512:1024], ps1[:])

            # ---- Stage B: per-channel chunked Toeplitz conv ----
            with (
                tc.tile_pool(name="gsl", bufs=2) as gp,
                tc.tile_pool(name="ysb", bufs=2) as yp,
                tc.tile_pool(name="psB", bufs=8, space="PSUM") as psB,
            ):
                for c0 in range(0, C, YG):          # output staging block
                    y_sb = yp.tile([CH, B_LOC * ND, YG], bf)
                    for cg0 in range(c0, c0 + YG, CG):   # G slab group
                        nd_grp = max(nd_c[cg0:cg0 + CG])
                        gsl = gp.tile([CH, ND, CG, CH], bf)
                        for d in range(nd_grp):
                            nc.gpsimd.dma_start(
                                out=gsl[:, d, :, :],
                                in_=g_in[d, :, cg0:cg0 + CG, :],
                            )
                        for c4 in range(0, CG, 4):
                            # psum columns are j-major: col = j*B_LOC + b
                            ps = psB.tile([128, 4, ND * B_LOC], f32)
                            for ci in range(4):
                                c = cg0 + c4 + ci
                                nd = nd_c[c]
                                for d in range(nd):
                                    nc.tensor.matmul(
                                        ps[:, ci, d * B_LOC:ND * B_LOC],
                                        gsl[:, d, (c4 + ci), :],
                                        v_sb[:, 0:(ND - d) * B_LOC, c],
                                        start=(d == 0),
                                        stop=(d == nd - 1),
                                    )
                            # relu + cast, psum (t, ci, (j b)) -> y_sb (t, (b j), c)
                            # alternate between DVE and ACT to double
                            # drain throughput (both are AP-walk bound)
                            co = cg0 - c0 + c4
                            out_ap = y_sb[:, :, co:co + 4].rearrange(
                                "t (b j) ci -> t ci b j", j=ND)
                            in_ap = ps[:].rearrange(
                                "t ci (j b) -> t ci b j", b=B_LOC)
                            if (cg0 // CG * (CG // 4) + c4 // 4) % 2 == 0:
                                nc.vector.tensor_scalar_max(
                                    out_ap, in_ap, 0.0)
                            else:
                                nc.scalar.activation(
                                    out_ap, in_ap,
                                    mybir.ActivationFunctionType.Relu)
                    # y_sb (t, (b j), c) -> y[(b, j*128+t), c0:c0+YG]
                    nc.scalar.dma_start(
                        out=y_out[:, c0:c0 + YG].rearrange(
                            "(b j t) c -> t (b j) c", t=CH, j=ND),
                        in_=y_sb[:],
                    )

    nc.finalize()
    return nc


def _make_runner(nd_c=None):
    """Build the Bass kernel and a cached jitted SPMD callable."""
    import jax
    import jax.numpy as jnp
    from jax.sharding import Mesh, PartitionSpec as P, NamedSharding
    from jax.experimental.shard_map import shard_map
    from concourse import bass2jax
    from concourse.bass2jax import (
        _bass_exec_p, install_neuronx_cc_hook, partition_id_tensor)

    install_neuronx_cc_hook()
    nc = _build_bass(nd_c)

    devs = jax.devices()[:NCORES]
    mesh = Mesh(np.asarray(devs), ("core",))

    out_aval = jax.core.ShapedArray((R_LOC, C), jnp.bfloat16)

    def _body(xb, wct, g, yz):
        outs = _bass_exec_p.bind(
            xb, wct, g, yz, partition_id_tensor(),
            out_avals=(out_aval,),
            in_names=("x", "wct", "g", "y", "partition_id"),
            out_names=("y",),
            lowering_input_output_aliases=(),
            sim_require_finite=False,
            sim_require_nnan=False,
            nc=nc,
        )
        return outs[0]

    mapped = jax.jit(shard_map(
        _body, mesh=mesh,
        in_specs=(P("core"),) * 4,
        out_specs=P("core"),
        check_rep=False,
    ), donate_argnums=(3,), keep_unused=True)

    zero_fn = jax.jit(
        lambda: jnp.zeros((B * T, C), jnp.bfloat16),
        out_shardings=NamedSharding(mesh, P("core")))
    return mapped, mesh, zero_fn


def _get_state(W_proj, W_values, gain, pow_, line):
    import jax
    from jax.sharding import PartitionSpec as P, NamedSharding

    h = hashlib.md5()
    for a in (W_proj, W_values, gain, pow_, line):
        h.update(np.ascontiguousarray(a).tobytes())
    key = h.hexdigest()
    if key in _cache:
        return _cache[key]

    # per-channel kept diagonal-block count: drop tail blocks whose
    # biggest kernel value is < 1e-4 of the channel peak (z[c,0] = 1)
    p = 2.0 + _sigmoid(np.asarray(pow_, np.float64).reshape(C)) * 100.0
    ln = np.asarray(line, np.float64).reshape(T)
    nd_c = []
    for c in range(C):
        nd = 1
        for d in range(1, ND):
            if ln[CH * d - 127] ** p[c] >= 1e-4:
                nd = d + 1
        nd_c.append(nd)

    rkey = ("runner", tuple(nd_c))
    if rkey not in _cache:
        _cache[rkey] = _make_runner(nd_c)
    mapped, mesh, zero_fn = _cache[rkey]

    WcT, G = _build_consts(
        np.asarray(W_proj, np.float32), np.asarray(W_values, np.float32),
        np.asarray(gain, np.float32), np.asarray(pow_, np.float32),
        np.asarray(line, np.float32))
    # stack per-core copies on axis 0 so each shard is exactly the
    # per-core BIR shape (no reshape inside the partitioned body)
    sh = NamedSharding(mesh, P("core"))
    wct_d = jax.device_put(
        np.ascontiguousarray(np.broadcast_to(WcT, (NCORES,) + WcT.shape))
        .reshape(NCORES * WcT.shape[0], WcT.shape[1]), sh)
    g_d = jax.device_put(
        np.ascontiguousarray(np.broadcast_to(G, (NCORES,) + G.shape))
        .reshape(NCORES * G.shape[0], *G.shape[1:]), sh)
    state = (mapped, mesh, zero_fn, wct_d, g_d)
    _cache[key] = state
    return state


def kernel(x, W_proj, W_values, gain, pow_, line):
    import jax
    from jax.sharding import PartitionSpec as P, NamedSharding

    mapped, mesh, zero_fn, wct_d, g_d = _get_state(
        W_proj, W_values, gain, pow_, line)

    xb = np.asarray(x, np.float32).reshape(B * T, C).astype(ml_dtypes.bfloat16)
    xd = jax.device_put(xb, NamedSharding(mesh, P("core")))
    out = mapped(xd, wct_d, g_d, zero_fn())
    y = np.asarray(out).astype(np.float32)
    return y.reshape(B, T, C)
